# revision 2
# baseline (speedup 1.0000x reference)
"""Trainium2 Bass kernel for online forward-mode sensitivity propagation.

Math restructuring: the tangent recurrence for parameter direction p=(i,j)
is linear in (jx, jv) with forcing x_t[n,j] * e_i injected each step.  By
superposition over injection times,

    jac[n,d,i,j] = sum_t Kx(T-1-t)[i,d] * x_t[n,j]

where Kx(tau) is the impulse-response propagator: Kx(0)=dt^2*I, Kv(0)=dt*I,
Kv' = Kv + dt*Kx@(W^T - I), Kx' = Kx + dt*Kv'.  Working with Mx = Kx^T the
propagator recurrence becomes *identical* to the primal state recurrence
(left-multiplied by (W - I)), so one combined state S = [x^T | Mx],
Sv = [v^T | Mv] is propagated with one 64x80 matmul per step.  The Jacobian
then factorizes as, per sample row n,

    jac[n]  (as a [(d,i), j] = [4096, 64] matrix)  =  G @ H_n

with G[(d,i), t] = Mx(T-1-t)[d,i] and H_n[t, j] = x_t[n, j] — a rank-16
contraction instead of propagating 4096 tangent states for 16 steps.

Sharding: the 128 sample rows are split 16-per-core across 8 NeuronCores
(rows of x evolve independently); each core computes its own (replicated,
tiny) propagator chain and its 16.8 MB Jacobian shard.  Output writes are
fully contiguous 256 KB blocks.
"""

import numpy as np
from contextlib import ExitStack

DT = 0.01
NCORES = 8
N, D = 128, 64  # problem dims (hardcoded per spec)


def _build_nc(T, SE, NL):
    import concourse.bass as bass
    import concourse.tile as tile
    from concourse import bacc, mybir
    from concourse.masks import make_identity

    f32 = mybir.dt.float32
    dt = DT
    NF = T // SE + 1            # stored trajectory frames (incl. x0 and x_T)
    PB = T * NL                 # total (t, n) rows for the X reshape
    NB = PB // 128              # number of 128-row transpose blocks
    assert PB % 128 == 0
    NH = (NL * D) // 512        # big-matmul free-dim halves
    NCHUNK = (D * D) // 128     # (d,i) row chunks of G

    nc = bacc.Bacc(
        "TRN2",
        target_bir_lowering=False,
        debug=False,
        enable_asserts=False,
        num_devices=NCORES,
    )

    x0s = nc.dram_tensor("x0s", [NL, D], f32, kind="ExternalInput").ap()
    v0s = nc.dram_tensor("v0s", [NL, D], f32, kind="ExternalInput").ap()
    w_in = nc.dram_tensor("w", [D, D], f32, kind="ExternalInput").ap()
    jac_out = nc.dram_tensor("jac", [NL, D * D, D], f32, kind="ExternalOutput").ap()
    traj_out = nc.dram_tensor("traj", [NF, NL, D], f32, kind="ExternalOutput").ap()

    with tile.TileContext(nc) as tc, ExitStack() as ctx:
        const = ctx.enter_context(tc.tile_pool(name="const", bufs=1))
        chain = ctx.enter_context(tc.tile_pool(name="chain", bufs=3))
        acc = ctx.enter_context(tc.tile_pool(name="acc", bufs=1))
        psA = ctx.enter_context(tc.tile_pool(name="psA", bufs=2, space="PSUM"))
        psT = ctx.enter_context(tc.tile_pool(name="psT", bufs=2, space="PSUM"))
        psB = ctx.enter_context(tc.tile_pool(name="psB", bufs=4, space="PSUM"))
        osb = ctx.enter_context(tc.tile_pool(name="osb", bufs=4))
        dram = ctx.enter_context(tc.tile_pool(name="dram", bufs=1, space="DRAM"))

        mult = mybir.AluOpType.mult
        add = mybir.AluOpType.add

        # ---- setup: load inputs, identity, W^T, W^T - I ----
        xin = const.tile([NL, D], f32)
        nc.sync.dma_start(xin[:], x0s)
        vin = const.tile([NL, D], f32)
        nc.sync.dma_start(vin[:], v0s)
        wsb = const.tile([D, D], f32)
        nc.sync.dma_start(wsb[:], w_in)
        ident = const.tile([128, 128], f32)
        make_identity(nc, ident[:])

        ps_wt = psT.tile([D, D], f32, tag="pst")
        nc.tensor.transpose(ps_wt[:], wsb[:], ident[0:D, 0:D])
        wmi = const.tile([D, D], f32)  # W^T - I  (lhsT for the chain matmul)
        nc.vector.scalar_tensor_tensor(
            wmi[:], ident[0:D, 0:D], -1.0, ps_wt[:], op0=mult, op1=add
        )

        # ---- initial combined state S0 = [x0^T | dt^2 I], Sv0 = [v0^T | dt I] ----
        S0 = chain.tile([D, NL + D], f32, tag="S")
        Sv0 = chain.tile([D, NL + D], f32, tag="Sv")
        ps_x0t = psT.tile([D, NL], f32, tag="pst")
        nc.tensor.transpose(ps_x0t[:], xin[:], ident[0:NL, 0:NL])
        nc.vector.tensor_copy(S0[:, 0:NL], ps_x0t[:])
        ps_v0t = psT.tile([D, NL], f32, tag="pst")
        nc.tensor.transpose(ps_v0t[:], vin[:], ident[0:NL, 0:NL])
        nc.vector.tensor_copy(Sv0[:, 0:NL], ps_v0t[:])
        nc.scalar.mul(S0[:, NL:NL + D], ident[0:D, 0:D], dt * dt)
        nc.scalar.mul(Sv0[:, NL:NL + D], ident[0:D, 0:D], dt)

        # ---- accumulators for the reshape stage ----
        Y = acc.tile([D, PB], f32)            # Y[:, t*NL+n] = x_t^T column n
        G_acc = acc.tile([D, T * D], f32)     # G_acc[:, (T-1-t)*D + i] = Mx(t)[:, i]

        # ---- the chain: 16 steps, 1 matmul + 2 fused DVE ops each ----
        S, Sv = S0, Sv0
        for t in range(T):
            nc.vector.tensor_copy(Y[:, t * NL:(t + 1) * NL], S[:, 0:NL])
            nc.vector.tensor_copy(
                G_acc[:, (T - 1 - t) * D:(T - t) * D], S[:, NL:NL + D]
            )
            ps = psA.tile([D, NL + D], f32)
            nc.tensor.matmul(ps[:], wmi[:], S[:], start=True, stop=True)
            Sv_n = chain.tile([D, NL + D], f32, tag="Sv")
            nc.vector.scalar_tensor_tensor(
                Sv_n[:], ps[:], dt, Sv[:], op0=mult, op1=add
            )
            S_n = chain.tile([D, NL + D], f32, tag="S")
            nc.vector.scalar_tensor_tensor(
                S_n[:], Sv_n[:], dt, S[:], op0=mult, op1=add
            )
            S, Sv = S_n, Sv_n

        # ---- reshape stage: transposes + DRAM roundtrip ----
        Gscratch = dram.tile([T, D, D], f32)
        nc.sync.dma_start(
            Gscratch[:].rearrange("t d i -> d t i"),
            G_acc[:].rearrange("d (t i) -> d t i", t=T),
        )

        Xscratch = dram.tile([T, NL, D], f32)
        xs_view = Xscratch[:].rearrange("(b r) n j -> b (r n) j", b=NB)
        for b in range(NB):
            ps_tb = psT.tile([128, D], f32, tag="pst")
            nc.tensor.transpose(ps_tb[:], Y[:, b * 128:(b + 1) * 128], ident[0:D, 0:D])
            tb = osb.tile([128, D], f32, tag="tb_sb")
            nc.vector.tensor_copy(tb[:], ps_tb[:])
            nc.sync.dma_start(xs_view[b], tb[:])
            # trajectory frames living in this block
            for f in range(NF - 1):
                t = f * SE
                if b * 128 <= t * NL < (b + 1) * 128:
                    off = t * NL - b * 128
                    nc.sync.dma_start(traj_out[f], tb[off:off + NL, :])

        # final frame x_T from the last chain state
        ps_xT = psT.tile([NL, D], f32, tag="pst")
        nc.tensor.transpose(ps_xT[:], S[:, 0:NL], ident[0:D, 0:D])
        xT_sb = osb.tile([NL, D], f32, tag="xT_sb")
        nc.vector.tensor_copy(xT_sb[:], ps_xT[:])
        nc.sync.dma_start(traj_out[NF - 1], xT_sb[:])

        # ---- readback: G with t on partitions, X with t on partitions ----
        Gsb = acc.tile([T, D * D], f32)
        nc.sync.dma_start(Gsb[:], Gscratch[:].rearrange("t d i -> t (d i)"))
        Xall = acc.tile([T, NL * D], f32)
        nc.sync.dma_start(Xall[:], Xscratch[:].rearrange("t n j -> t (n j)"))

        # ---- big stage: jac[(d,i) chunk, (n,j) half] = G_chunk^T... @ X ----
        NPH = 512 // D  # samples per free-dim half
        jac_v = jac_out.rearrange("n r j -> r n j")
        for c in range(NCHUNK):
            for h in range(NH):
                ps = psB.tile([128, 512], f32)
                nc.tensor.matmul(
                    ps[:],
                    Gsb[:, c * 128:(c + 1) * 128],
                    Xall[:, h * 512:(h + 1) * 512],
                    start=True,
                    stop=True,
                )
                ot = osb.tile([128, 512], f32, tag="ot")
                nc.vector.tensor_copy(ot[:], ps[:])
                nc.sync.dma_start(
                    jac_v[c * 128:(c + 1) * 128, h * NPH:(h + 1) * NPH, :],
                    ot[:].rearrange("r (n j) -> r n j", n=NPH),
                )

    nc.compile()
    return nc


_cache = {}


def _get_nc(T, SE, NL):
    key = (T, SE, NL)
    if key not in _cache:
        _cache[key] = _build_nc(T, SE, NL)
    return _cache[key]


def kernel(x0, v0, force_weight, n_steps, store_every, _trace=False):
    from concourse.bass_utils import run_bass_kernel_spmd

    x0 = np.asarray(x0, dtype=np.float32)
    v0 = np.asarray(v0, dtype=np.float32)
    W = np.asarray(force_weight, dtype=np.float32)
    T = int(n_steps)
    SE = int(store_every)
    n, d = x0.shape
    assert (n, d) == (N, D)
    NL = n // NCORES
    NF = T // SE + 1

    nc = _get_nc(T, SE, NL)
    in_maps = [
        {
            "x0s": np.ascontiguousarray(x0[k * NL:(k + 1) * NL]),
            "v0s": np.ascontiguousarray(v0[k * NL:(k + 1) * NL]),
            "w": np.ascontiguousarray(W),
        }
        for k in range(NCORES)
    ]
    res = run_bass_kernel_spmd(
        nc, in_maps, core_ids=list(range(NCORES)), trace=_trace
    )
    kernel.last_results = res

    jac = np.concatenate(
        [res.results[k]["jac"].reshape(NL, D, D, D) for k in range(NCORES)], axis=0
    )
    traj = np.concatenate(
        [res.results[k]["traj"] for k in range(NCORES)], axis=1
    )
    assert traj.shape == (NF, n, d) and jac.shape == (n, d, D, D)
    return traj, jac


# revision 3
# speedup vs baseline: 1.1543x; 1.1543x over previous
"""Trainium2 Bass kernel for online forward-mode sensitivity propagation.

Math restructuring: the tangent recurrence for parameter direction p=(i,j)
is linear in (jx, jv) with forcing x_t[n,j] * e_i injected each step.  By
superposition over injection times,

    jac[n,d,i,j] = sum_t Kx(T-1-t)[i,d] * x_t[n,j]

where Kx(tau) is the impulse-response propagator: Kx(0)=dt^2*I, Kv(0)=dt*I,
Kv' = Kv + dt*Kx@(W^T - I), Kx' = Kx + dt*Kv'.  Working with Mx = Kx^T the
propagator recurrence becomes *identical* to the primal state recurrence
(left-multiplied by (W - I)), so one combined state S = [x^T | Mx],
Sv = [v^T | Mv] is propagated with one 64x80 matmul per step.  The Jacobian
then factorizes as, per sample row n,

    jac[n]  (as a [(d,i), j] = [4096, 64] matrix)  =  G @ H_n

with G[(d,i), t] = Mx(T-1-t)[d,i] and H_n[t, j] = x_t[n, j] — a rank-16
contraction instead of propagating 4096 tangent states for 16 steps.

The big contraction runs as bf16 matmuls with a hi/lo split (3 accumulating
passes: Gh@Xh + Gl@Xh + Gh@Xl, fp32 PSUM) — ~1e-5 relative error, ~3x
faster than the PE's fp32 LOW_HIGH mode.

Sharding: the 128 sample rows are split 16-per-core across 8 NeuronCores
(rows of x evolve independently); each core computes its own (replicated,
tiny) propagator chain and its 16.8 MB Jacobian shard.  Output writes are
fully contiguous 256 KB blocks.
"""

import numpy as np
from contextlib import ExitStack

DT = 0.01
NCORES = 8
N, D = 128, 64  # problem dims (hardcoded per spec)


def _build_nc(T, SE, NL):
    import concourse.bass as bass
    import concourse.tile as tile
    from concourse import bacc, mybir
    from concourse.masks import make_identity

    f32 = mybir.dt.float32
    bf16 = mybir.dt.bfloat16
    dt = DT
    NF = T // SE + 1            # stored trajectory frames (incl. x0 and x_T)
    PB = T * NL                 # total (t, n) rows for the X reshape
    NB = PB // 128              # number of 128-row transpose blocks
    assert PB % 128 == 0
    NH = (NL * D) // 512        # big-matmul free-dim halves
    NCHUNK = (D * D) // 128     # (d,i) row chunks of G
    TPB = T // NB               # chain steps per transpose block

    nc = bacc.Bacc(
        "TRN2",
        target_bir_lowering=False,
        debug=False,
        enable_asserts=False,
        num_devices=NCORES,
    )

    x0s = nc.dram_tensor("x0s", [NL, D], f32, kind="ExternalInput").ap()
    v0s = nc.dram_tensor("v0s", [NL, D], f32, kind="ExternalInput").ap()
    w_in = nc.dram_tensor("w", [D, D], f32, kind="ExternalInput").ap()
    jac_out = nc.dram_tensor("jac", [NL, D * D, D], f32, kind="ExternalOutput").ap()
    traj_out = nc.dram_tensor("traj", [NF, NL, D], f32, kind="ExternalOutput").ap()

    with tile.TileContext(nc) as tc, ExitStack() as ctx:
        const = ctx.enter_context(tc.tile_pool(name="const", bufs=1))
        chain = ctx.enter_context(tc.tile_pool(name="chain", bufs=3))
        acc = ctx.enter_context(tc.tile_pool(name="acc", bufs=1))
        psA = ctx.enter_context(tc.tile_pool(name="psA", bufs=2, space="PSUM"))
        psT = ctx.enter_context(tc.tile_pool(name="psT", bufs=2, space="PSUM"))
        psB = ctx.enter_context(tc.tile_pool(name="psB", bufs=4, space="PSUM"))
        osb = ctx.enter_context(tc.tile_pool(name="osb", bufs=4))
        dram = ctx.enter_context(tc.tile_pool(name="dram", bufs=1, space="DRAM"))

        mult = mybir.AluOpType.mult
        add = mybir.AluOpType.add

        # ---- setup: load inputs, identity, W^T, W^T - I ----
        xin = const.tile([NL, D], f32)
        nc.sync.dma_start(xin[:], x0s)
        vin = const.tile([NL, D], f32)
        nc.sync.dma_start(vin[:], v0s)
        wsb = const.tile([D, D], f32)
        nc.sync.dma_start(wsb[:], w_in)
        ident = const.tile([128, 128], f32)
        make_identity(nc, ident[:])

        ps_wt = psT.tile([D, D], f32, tag="pst")
        nc.tensor.transpose(ps_wt[:], wsb[:], ident[0:D, 0:D])
        wmi = const.tile([D, D], f32)  # W^T - I  (lhsT for the chain matmul)
        nc.vector.scalar_tensor_tensor(
            wmi[:], ident[0:D, 0:D], -1.0, ps_wt[:], op0=mult, op1=add
        )

        # ---- initial combined state S0 = [x0^T | dt^2 I], Sv0 = [v0^T | dt I] ----
        S0 = chain.tile([D, NL + D], f32, tag="S")
        Sv0 = chain.tile([D, NL + D], f32, tag="Sv")
        ps_x0t = psT.tile([D, NL], f32, tag="pst")
        nc.tensor.transpose(ps_x0t[:], xin[:], ident[0:NL, 0:NL])
        nc.vector.tensor_copy(S0[:, 0:NL], ps_x0t[:])
        ps_v0t = psT.tile([D, NL], f32, tag="pst")
        nc.tensor.transpose(ps_v0t[:], vin[:], ident[0:NL, 0:NL])
        nc.vector.tensor_copy(Sv0[:, 0:NL], ps_v0t[:])
        nc.scalar.mul(S0[:, NL:NL + D], ident[0:D, 0:D], dt * dt)
        nc.scalar.mul(Sv0[:, NL:NL + D], ident[0:D, 0:D], dt)

        # ---- accumulators for the reshape stage ----
        Y = acc.tile([D, PB], f32)            # Y[:, t*NL+n] = x_t^T column n
        G_acc = acc.tile([D, T * D], f32)     # G_acc[:, (T-1-t)*D + i] = Mx(t)[:, i]

        # scratch + on-chip tiles for the reshape stage
        Gscratch = dram.tile([T, D, D], f32)
        Xscratch = dram.tile([T, NL, D], f32)
        xs_view = Xscratch[:].rearrange("(b r) n j -> b (r n) j", b=NB)
        Gsb = acc.tile([T, D * D], f32)
        Xall = acc.tile([T, NL * D], f32)

        def emit_block_reshape(b):
            """Transpose Y block b -> natural-layout x rows, store + traj frames."""
            ps_tb = psT.tile([128, D], f32, tag="pst")
            nc.tensor.transpose(
                ps_tb[:], Y[:, b * 128:(b + 1) * 128], ident[0:D, 0:D]
            )
            tb = osb.tile([128, D], f32, tag="tb_sb")
            nc.vector.tensor_copy(tb[:], ps_tb[:])
            nc.sync.dma_start(xs_view[b], tb[:])
            for f in range(NF - 1):
                t = f * SE
                if b * 128 <= t * NL < (b + 1) * 128:
                    off = t * NL - b * 128
                    nc.sync.dma_start(traj_out[f], tb[off:off + NL, :])
            # readback of this block's X rows with t on partitions
            rows = slice(b * TPB, (b + 1) * TPB)
            nc.scalar.dma_start(
                Xall[rows, :],
                Xscratch[:].rearrange("t n j -> t (n j)")[rows, :],
            )

        def emit_g_half(lohi):
            """Store + read back half of G (row range in t-index space)."""
            rows = slice(lohi * (T // 2), (lohi + 1) * (T // 2))
            nc.sync.dma_start(
                Gscratch[rows].rearrange("t d i -> d t i"),
                G_acc[:, rows.start * D:rows.stop * D].rearrange(
                    "d (t i) -> d t i", t=T // 2
                ),
            )
            nc.scalar.dma_start(
                Gsb[rows, :],
                Gscratch[:].rearrange("t d i -> t (d i)")[rows, :],
            )

        # ---- the chain: T steps, 1 matmul + 2 fused DVE ops each ----
        S, Sv = S0, Sv0
        for t in range(T):
            nc.vector.tensor_copy(Y[:, t * NL:(t + 1) * NL], S[:, 0:NL])
            nc.vector.tensor_copy(
                G_acc[:, (T - 1 - t) * D:(T - t) * D], S[:, NL:NL + D]
            )
            if t == T // 2:
                # first half of G (t rows T/2..T-1 = Mx(0..T/2-1)) is final
                emit_g_half(1)
            ps = psA.tile([D, NL + D], f32)
            nc.tensor.matmul(ps[:], wmi[:], S[:], start=True, stop=True)
            Sv_n = chain.tile([D, NL + D], f32, tag="Sv")
            nc.vector.scalar_tensor_tensor(
                Sv_n[:], ps[:], dt, Sv[:], op0=mult, op1=add
            )
            S_n = chain.tile([D, NL + D], f32, tag="S")
            nc.vector.scalar_tensor_tensor(
                S_n[:], Sv_n[:], dt, S[:], op0=mult, op1=add
            )
            S, Sv = S_n, Sv_n
            if (t + 1) % TPB == 0:
                emit_block_reshape((t + 1) // TPB - 1)
        emit_g_half(0)

        # final frame x_T from the last chain state
        ps_xT = psT.tile([NL, D], f32, tag="pst")
        nc.tensor.transpose(ps_xT[:], S[:, 0:NL], ident[0:D, 0:D])
        xT_sb = osb.tile([NL, D], f32, tag="xT_sb")
        nc.vector.tensor_copy(xT_sb[:], ps_xT[:])
        nc.sync.dma_start(traj_out[NF - 1], xT_sb[:])

        # ---- hi/lo bf16 split of G and X ----
        G_hi = acc.tile([T, D * D], bf16)
        nc.vector.tensor_copy(G_hi[:], Gsb[:])
        G_rem = acc.tile([T, D * D], f32)
        nc.vector.scalar_tensor_tensor(
            G_rem[:], G_hi[:], -1.0, Gsb[:], op0=mult, op1=add
        )
        G_lo = acc.tile([T, D * D], bf16)
        nc.vector.tensor_copy(G_lo[:], G_rem[:])

        X_hi = acc.tile([T, NL * D], bf16)
        nc.vector.tensor_copy(X_hi[:], Xall[:])
        X_rem = acc.tile([T, NL * D], f32)
        nc.vector.scalar_tensor_tensor(
            X_rem[:], X_hi[:], -1.0, Xall[:], op0=mult, op1=add
        )
        X_lo = acc.tile([T, NL * D], bf16)
        nc.vector.tensor_copy(X_lo[:], X_rem[:])

        # ---- big stage: jac chunk = Gh.T@Xh + Gl.T@Xh + Gh.T@Xl ----
        NPH = 512 // D  # samples per free-dim half
        jac_v = jac_out.rearrange("n r j -> r n j")
        for c in range(NCHUNK):
            cs = slice(c * 128, (c + 1) * 128)
            for h in range(NH):
                hs = slice(h * 512, (h + 1) * 512)
                ps = psB.tile([128, 512], f32)
                nc.tensor.matmul(
                    ps[:], G_hi[:, cs], X_hi[:, hs], start=True, stop=False
                )
                nc.tensor.matmul(
                    ps[:], G_lo[:, cs], X_hi[:, hs], start=False, stop=False
                )
                nc.tensor.matmul(
                    ps[:], G_hi[:, cs], X_lo[:, hs], start=False, stop=True
                )
                ot = osb.tile([128, 512], f32, tag="ot")
                nc.vector.tensor_copy(ot[:], ps[:])
                eng = nc.sync if (c * NH + h) % 2 == 0 else nc.scalar
                eng.dma_start(
                    jac_v[cs, h * NPH:(h + 1) * NPH, :],
                    ot[:].rearrange("r (n j) -> r n j", n=NPH),
                )

    nc.compile()
    return nc


_cache = {}


def _get_nc(T, SE, NL):
    key = (T, SE, NL)
    if key not in _cache:
        _cache[key] = _build_nc(T, SE, NL)
    return _cache[key]


def kernel(x0, v0, force_weight, n_steps, store_every, _trace=False):
    from concourse.bass_utils import run_bass_kernel_spmd

    x0 = np.asarray(x0, dtype=np.float32)
    v0 = np.asarray(v0, dtype=np.float32)
    W = np.asarray(force_weight, dtype=np.float32)
    T = int(n_steps)
    SE = int(store_every)
    n, d = x0.shape
    assert (n, d) == (N, D)
    NL = n // NCORES
    NF = T // SE + 1

    nc = _get_nc(T, SE, NL)
    in_maps = [
        {
            "x0s": np.ascontiguousarray(x0[k * NL:(k + 1) * NL]),
            "v0s": np.ascontiguousarray(v0[k * NL:(k + 1) * NL]),
            "w": np.ascontiguousarray(W),
        }
        for k in range(NCORES)
    ]
    res = run_bass_kernel_spmd(
        nc, in_maps, core_ids=list(range(NCORES)), trace=_trace
    )
    kernel.last_results = res

    jac = np.concatenate(
        [res.results[k]["jac"].reshape(NL, D, D, D) for k in range(NCORES)], axis=0
    )
    traj = np.concatenate(
        [res.results[k]["traj"] for k in range(NCORES)], axis=1
    )
    assert traj.shape == (NF, n, d) and jac.shape == (n, d, D, D)
    return traj, jac


# revision 4
# speedup vs baseline: 1.5203x; 1.3170x over previous
"""Trainium2 Bass kernel for online forward-mode sensitivity propagation.

Math restructuring: the tangent recurrence for parameter direction p=(i,j)
is linear in (jx, jv) with forcing x_t[n,j] * e_i injected each step.  By
superposition over injection times,

    jac[n,d,i,j] = sum_t Kx(T-1-t)[i,d] * x_t[n,j]

where Kx(tau) is the impulse-response propagator: Kx(0)=dt^2*I, Kv(0)=dt*I,
Kv' = Kv + dt*Kx@(W^T - I), Kx' = Kx + dt*Kv'.  Working with Mx = Kx^T the
propagator recurrence becomes *identical* to the primal state recurrence
(left-multiplied by (W - I)), so one combined state S = [x^T | Mx],
Sv = [v^T | Mv] is propagated with one 64x80 matmul per step.  The Jacobian
then factorizes as, per sample row n,

    jac[n]  (as a [(d,i), j] = [4096, 64] matrix)  =  G @ H_n

with G[(d,i), t] = Mx(T-1-t)[d,i] and H_n[t, j] = x_t[n, j] — a rank-16
contraction instead of propagating 4096 tangent states for 16 steps.

Big-stage implementation notes:
- Operands are split hi/lo in bf16 and stacked along the contraction dim
  (K = 4*T = 64 rows: Gh|Gl|Gh|Gl against Xh|Xh|Xl|Xl), so one bf16 matmul
  computes the full (Gh+Gl)^T (Xh+Xl) product with fp32 PSUM accumulation
  (~1e-7 operand error) — ~4x faster than the PE's fp32 LOW_HIGH mode.
- Outputs are produced in (d,i)-row pairs: even rows via one matmul into
  bank A, odd rows via a second matmul into bank B, run CONCURRENTLY in
  different PE row-groups (tile_position (0,0)/(64,0) with replicated
  operands).  A single strided DVE copy interleaves the pair so the store
  DMA gets 512-byte contiguous runs (2x fewer, 2x larger descriptors).

Sharding: the 128 sample rows are split 16-per-core across 8 NeuronCores
(rows of x evolve independently); each core computes its own (replicated,
tiny) propagator chain and its 16.8 MB Jacobian shard.
"""

import numpy as np
from contextlib import ExitStack

DT = 0.01
NCORES = 8
N, D = 128, 64  # problem dims (hardcoded per spec)


def _build_nc(T, SE, NL):
    import concourse.bass as bass
    import concourse.tile as tile
    from concourse import bacc, mybir
    from concourse.masks import make_identity

    f32 = mybir.dt.float32
    bf16 = mybir.dt.bfloat16
    dt = DT
    NF = T // SE + 1            # stored trajectory frames (incl. x0 and x_T)
    PB = T * NL                 # total (t, n) rows for the X reshape
    NB = PB // 128              # number of 128-row transpose blocks
    assert PB % 128 == 0
    NH = (NL * D) // 512        # big-matmul free-dim halves
    NPH = 512 // D              # samples per free-dim half
    NG = (D * D) // 256         # 256-row output chunks
    TPB = T // NB               # chain steps per transpose block
    K = 4 * T                   # stacked contraction length

    nc = bacc.Bacc(
        "TRN2",
        target_bir_lowering=False,
        debug=False,
        enable_asserts=False,
        num_devices=NCORES,
    )

    x0s = nc.dram_tensor("x0s", [NL, D], f32, kind="ExternalInput").ap()
    v0s = nc.dram_tensor("v0s", [NL, D], f32, kind="ExternalInput").ap()
    w_in = nc.dram_tensor("w", [D, D], f32, kind="ExternalInput").ap()
    jac_out = nc.dram_tensor("jac", [NL, D * D, D], f32, kind="ExternalOutput").ap()
    traj_out = nc.dram_tensor("traj", [NF, NL, D], f32, kind="ExternalOutput").ap()

    with tile.TileContext(nc) as tc, ExitStack() as ctx:
        const = ctx.enter_context(tc.tile_pool(name="const", bufs=1))
        chain = ctx.enter_context(tc.tile_pool(name="chain", bufs=3))
        acc = ctx.enter_context(tc.tile_pool(name="acc", bufs=1))
        psA = ctx.enter_context(tc.tile_pool(name="psA", bufs=2, space="PSUM"))
        psT = ctx.enter_context(tc.tile_pool(name="psT", bufs=2, space="PSUM"))
        psB = ctx.enter_context(tc.tile_pool(name="psB", bufs=2, space="PSUM"))
        osb = ctx.enter_context(tc.tile_pool(name="osb", bufs=4))
        dram = ctx.enter_context(tc.tile_pool(name="dram", bufs=1, space="DRAM"))

        mult = mybir.AluOpType.mult
        add = mybir.AluOpType.add

        # ---- setup: load inputs, identity, W^T, W^T - I ----
        xin = const.tile([NL, D], f32)
        nc.sync.dma_start(xin[:], x0s)
        vin = const.tile([NL, D], f32)
        nc.sync.dma_start(vin[:], v0s)
        wsb = const.tile([D, D], f32)
        nc.sync.dma_start(wsb[:], w_in)
        ident = const.tile([128, 128], f32)
        make_identity(nc, ident[:])

        ps_wt = psT.tile([D, D], f32, tag="pst")
        nc.tensor.transpose(ps_wt[:], wsb[:], ident[0:D, 0:D])
        wmi = const.tile([D, D], f32)  # W^T - I  (lhsT for the chain matmul)
        nc.vector.scalar_tensor_tensor(
            wmi[:], ident[0:D, 0:D], -1.0, ps_wt[:], op0=mult, op1=add
        )

        # ---- initial combined state S0 = [x0^T | dt^2 I], Sv0 = [v0^T | dt I] ----
        S0 = chain.tile([D, NL + D], f32, tag="S")
        Sv0 = chain.tile([D, NL + D], f32, tag="Sv")
        ps_x0t = psT.tile([D, NL], f32, tag="pst")
        nc.tensor.transpose(ps_x0t[:], xin[:], ident[0:NL, 0:NL])
        nc.vector.tensor_copy(S0[:, 0:NL], ps_x0t[:])
        ps_v0t = psT.tile([D, NL], f32, tag="pst")
        nc.tensor.transpose(ps_v0t[:], vin[:], ident[0:NL, 0:NL])
        nc.vector.tensor_copy(Sv0[:, 0:NL], ps_v0t[:])
        nc.scalar.mul(S0[:, NL:NL + D], ident[0:D, 0:D], dt * dt)
        nc.scalar.mul(Sv0[:, NL:NL + D], ident[0:D, 0:D], dt)

        # ---- accumulators + scratch ----
        Y = acc.tile([D, PB], f32)            # Y[:, t*NL+n] = x_t^T column n
        G_acc = acc.tile([D, T * D], f32)     # G_acc[:, (T-1-t)*D + i] = Mx(t)[:, i]
        Gs = dram.tile([2, T, D, D], bf16)    # (hi/lo, t, d, i)
        Xs = dram.tile([2, T, NL, D], bf16)   # (hi/lo, t, n, j)
        xsv = [Xs[hl].rearrange("(b r) n j -> b (r n) j", b=NB) for hl in (0, 1)]

        def emit_block_reshape(b):
            """Transpose Y block b -> natural-layout x rows; traj + hi/lo store."""
            ps_tb = psT.tile([128, D], f32, tag="pst")
            nc.tensor.transpose(
                ps_tb[:], Y[:, b * 128:(b + 1) * 128], ident[0:D, 0:D]
            )
            tb = osb.tile([128, D], f32, tag="tb_sb")
            nc.vector.tensor_copy(tb[:], ps_tb[:])
            for f in range(NF - 1):
                t = f * SE
                if b * 128 <= t * NL < (b + 1) * 128:
                    off = t * NL - b * 128
                    nc.sync.dma_start(traj_out[f], tb[off:off + NL, :])
            tbh = osb.tile([128, D], bf16, tag="tbh")
            nc.vector.tensor_copy(tbh[:], tb[:])
            tbr = osb.tile([128, D], f32, tag="tbr")
            nc.vector.scalar_tensor_tensor(
                tbr[:], tbh[:], -1.0, tb[:], op0=mult, op1=add
            )
            tbl = osb.tile([128, D], bf16, tag="tbl")
            nc.vector.tensor_copy(tbl[:], tbr[:])
            nc.sync.dma_start(xsv[0][b], tbh[:])
            nc.sync.dma_start(xsv[1][b], tbl[:])

        # ---- the chain: T steps, 1 matmul + 2 fused DVE ops each ----
        S, Sv = S0, Sv0
        for t in range(T):
            nc.vector.tensor_copy(Y[:, t * NL:(t + 1) * NL], S[:, 0:NL])
            nc.vector.tensor_copy(
                G_acc[:, (T - 1 - t) * D:(T - t) * D], S[:, NL:NL + D]
            )
            ps = psA.tile([D, NL + D], f32)
            nc.tensor.matmul(ps[:], wmi[:], S[:], start=True, stop=True)
            Sv_n = chain.tile([D, NL + D], f32, tag="Sv")
            nc.vector.scalar_tensor_tensor(
                Sv_n[:], ps[:], dt, Sv[:], op0=mult, op1=add
            )
            S_n = chain.tile([D, NL + D], f32, tag="S")
            nc.vector.scalar_tensor_tensor(
                S_n[:], Sv_n[:], dt, S[:], op0=mult, op1=add
            )
            S, Sv = S_n, Sv_n
            if (t + 1) % TPB == 0:
                emit_block_reshape((t + 1) // TPB - 1)

        # final frame x_T from the last chain state
        ps_xT = psT.tile([NL, D], f32, tag="pst")
        nc.tensor.transpose(ps_xT[:], S[:, 0:NL], ident[0:D, 0:D])
        xT_sb = osb.tile([NL, D], f32, tag="xT_sb")
        nc.vector.tensor_copy(xT_sb[:], ps_xT[:])
        nc.sync.dma_start(traj_out[NF - 1], xT_sb[:])

        # ---- hi/lo split of G, store to scratch ----
        Gh = acc.tile([D, T * D], bf16)
        nc.vector.tensor_copy(Gh[:], G_acc[:])
        Gr = acc.tile([D, T * D], f32)
        nc.vector.scalar_tensor_tensor(
            Gr[:], Gh[:], -1.0, G_acc[:], op0=mult, op1=add
        )
        Gl = acc.tile([D, T * D], bf16)
        nc.vector.tensor_copy(Gl[:], Gr[:])
        nc.sync.dma_start(
            Gs[0].rearrange("t d i -> d t i"),
            Gh[:].rearrange("d (t i) -> d t i", t=T),
        )
        nc.sync.dma_start(
            Gs[1].rearrange("t d i -> d t i"),
            Gl[:].rearrange("d (t i) -> d t i", t=T),
        )

        # ---- build stacked operands (x2 row-group replicas) ----
        # G stack rows: [Gh(T) | Gl(T) | Gh(T) | Gl(T)]  (as (hl t) twice)
        # X stack rows: [Xh(T) | Xh(T) | Xl(T) | Xl(T)]
        Gfat = acc.tile([128, D * D], bf16)
        Xfat = acc.tile([128, NL * D], bf16)
        gs_flat = Gs[:].rearrange("hl t d i -> (hl t) (d i)")
        for rep in range(2):
            for half in range(2):
                nc.scalar.dma_start(
                    Gfat[rep * K + half * 2 * T:rep * K + (half + 1) * 2 * T, :],
                    gs_flat,
                )
        for rep in range(2):
            for k4 in range(4):
                hl = k4 // 2
                nc.scalar.dma_start(
                    Xfat[rep * K + k4 * T:rep * K + (k4 + 1) * T, :],
                    Xs[hl].rearrange("t n j -> t (n j)"),
                )

        # ---- big stage: paired even/odd chunks in concurrent row groups ----
        jac_pair = jac_out.rearrange("n (gq p2) j -> gq n (p2 j)", p2=2)
        m = 0
        for g in range(NG):
            for h in range(NH):
                hs = slice(h * 512, (h + 1) * 512)
                ps = psB.tile([128, 1024], f32)
                nc.tensor.matmul(
                    ps[:, 0:512],
                    Gfat[0:K, g * 256:(g + 1) * 256:2],
                    Xfat[0:K, hs],
                    start=True, stop=True,
                )
                nc.tensor.matmul(
                    ps[:, 512:1024],
                    Gfat[K:2 * K, g * 256 + 1:(g + 1) * 256:2],
                    Xfat[K:2 * K, hs],
                    start=True, stop=True,
                    tile_position=(64, 0),
                )
                ot = osb.tile([128, 1024], f32, tag="ot")
                ov = ot[:].rearrange("q (n p2 j) -> q n p2 j", n=NPH, p2=2)
                iv = ps[:].rearrange("q (p2 n j) -> q n p2 j", p2=2, n=NPH)
                if m % 3 == 2:
                    nc.scalar.copy(ov, iv)
                else:
                    nc.vector.tensor_copy(ov, iv)
                nc.sync.dma_start(
                    jac_pair[g * 128:(g + 1) * 128, h * NPH:(h + 1) * NPH, :],
                    ot[:].rearrange("q (n p2 j) -> q n (p2 j)", n=NPH, p2=2),
                )
                m += 1

    nc.compile()
    return nc


_cache = {}


def _get_nc(T, SE, NL):
    key = (T, SE, NL)
    if key not in _cache:
        _cache[key] = _build_nc(T, SE, NL)
    return _cache[key]


def kernel(x0, v0, force_weight, n_steps, store_every, _trace=False):
    from concourse.bass_utils import run_bass_kernel_spmd

    x0 = np.asarray(x0, dtype=np.float32)
    v0 = np.asarray(v0, dtype=np.float32)
    W = np.asarray(force_weight, dtype=np.float32)
    T = int(n_steps)
    SE = int(store_every)
    n, d = x0.shape
    assert (n, d) == (N, D)
    NL = n // NCORES
    NF = T // SE + 1

    nc = _get_nc(T, SE, NL)
    in_maps = [
        {
            "x0s": np.ascontiguousarray(x0[k * NL:(k + 1) * NL]),
            "v0s": np.ascontiguousarray(v0[k * NL:(k + 1) * NL]),
            "w": np.ascontiguousarray(W),
        }
        for k in range(NCORES)
    ]
    res = run_bass_kernel_spmd(
        nc, in_maps, core_ids=list(range(NCORES)), trace=_trace
    )
    kernel.last_results = res

    jac = np.concatenate(
        [res.results[k]["jac"].reshape(NL, D, D, D) for k in range(NCORES)], axis=0
    )
    traj = np.concatenate(
        [res.results[k]["traj"] for k in range(NCORES)], axis=1
    )
    assert traj.shape == (NF, n, d) and jac.shape == (n, d, D, D)
    return traj, jac
